# revision 20
# baseline (speedup 1.0000x reference)
"""Trainium2 Bass kernel for nn_Attention (Bahdanau-style additive attention).

Reference computation:
    enc = encoder_outputs.transpose(1, 0, 2)            # [B, S, 2H]
    e_proj = enc @ w_e.T                                # [B, S, H]
    energy = tanh(h_proj[:, None, :] + e_proj + b)      # [B, S, H]
    att = energy @ v_w                                  # [B, S]
    out = softmax(att, axis=1)

Sharding: data-parallel over batch, 4 batch rows per core on 8 cores.

Per-core pipeline, |v|-stratified mixed precision: the logit error from
quantizing the e_proj GEMM is sum_h v_h * tanh' * dx_h, so the h
columns are permuted by descending |v_h| (host side) and the HOT
highest-|v| columns are computed in fp16 while the remaining COLD
columns run entirely in fp8 (e4m3) DoubleRow matmuls at 2x PE
throughput.  This buys the same accuracy as a chunk-wise fp16/fp8
split at ~12% less PE time.
  - enc is pre-transposed and quantized on the host into partition-major
    [e, chunk, s] lines (both dtypes cover all 16 contraction chunks),
    streamed per half batch row; all DMA is plain loads, no transpose
  - w_e and c_b are pre-scaled by WS=64 so the fp8 weights stay in the
    e4m3 normal range; the tanh activation applies scale=1/WS to undo it
  - LdWeights is split from Matmult so the next stationary (an enc
    chunk) loads while the current moving phase streams
  - epilogue per 128-position s-tile: DVE adds the broadcast bias
    c_b*WS per psum region, ACT applies tanh(x/WS), GPSIMD (otherwise
    idle) multiplies by v and reduces over h into the logit column
h_proj ([32,1024] @ [1024,1024]) and the final softmax over [32, 2048]
are tiny and run on the host in fp32.
"""

import sys

try:
    import concourse.bass as bass  # noqa: F401
except ImportError:
    sys.path.insert(0, "/opt/trn_rl_repo")

import numpy as np
import ml_dtypes

import concourse.bacc as bacc
import concourse.mybir as mybir
import concourse.tile as tile
from concourse.bass_utils import run_bass_kernel_spmd

HID = 1024
BATCH = 32
SRC_LEN = 2048

N_CORES = 8
B_LOC = BATCH // N_CORES      # 4
E = 2 * HID                   # 2048
N_EC = E // 128               # 16 e-chunks of 128
N_DR = N_EC // 2              # 8 fp8 DoubleRow chunk-pairs
HOT = 160                     # fp16 h-columns (highest |v|), permuted first
COLD = HID - HOT              # 832 fp8 h-columns
C0 = 512                      # cold psum region split: 512 + 320
C1 = COLD - C0                # 320
SH = SRC_LEN // 2             # 1024 s per half-row pipeline stage
N_STH = SH // 128             # 8 s-tiles per half
WS = 64.0                     # weight/bias pre-scale (fp8 range)

f32 = mybir.dt.float32
fp16 = mybir.dt.float16
fp8 = mybir.dt.float8e4

_NC_CACHE = {}


def _build():
    nc = bacc.Bacc(
        "TRN2", target_bir_lowering=False, debug=False, num_devices=N_CORES
    )
    # half-major layout: one fully-contiguous 32/16KB line per partition
    # per half, so each half load is descriptor-light on the DMA engines
    enc16 = nc.declare_dram_parameter(
        "enc16", [B_LOC, 2, 128, N_EC, SH], fp16, isOutput=False
    )
    enc8 = nc.declare_dram_parameter(
        "enc8", [B_LOC, 2, 128, N_EC, SH], fp8, isOutput=False
    )
    w16 = nc.declare_dram_parameter("w16", [128, N_EC, HOT], fp16, isOutput=False)
    w8 = nc.declare_dram_parameter("w8", [128, N_EC, COLD], fp8, isOutput=False)
    cbb = nc.declare_dram_parameter("cbb", [B_LOC, 128, HID], f32, isOutput=False)
    vb = nc.declare_dram_parameter("vb", [128, HID], fp16, isOutput=False)
    # [b, p, st]: logit(b, st*128 + p) in permuted-h space (h only summed)
    att = nc.declare_dram_parameter(
        "att", [B_LOC, 128, SRC_LEN // 128], f32, isOutput=True
    )

    with tile.TileContext(nc) as tc:
        with (
            tc.tile_pool(name="const", bufs=1) as const_pool,
            tc.tile_pool(name="e16p", bufs=2) as e16_pool,
            tc.tile_pool(name="e8p", bufs=2) as e8_pool,
            tc.tile_pool(name="cbbp", bufs=2) as cbb_pool,
            tc.tile_pool(name="prep", bufs=4) as pre_pool,
            tc.tile_pool(name="tep", bufs=3) as te_pool,
            tc.tile_pool(name="ttp", bufs=2) as tt_pool,
            tc.tile_pool(name="attsb", bufs=1) as att_pool,
            tc.tile_pool(name="psum", bufs=2, space="PSUM") as psum_pool,
        ):
            w16_sb = const_pool.tile([128, N_EC, HOT], fp16)
            w8_sb = const_pool.tile([128, N_EC, COLD], fp8)
            vb_sb = const_pool.tile([128, HID], fp16)
            att_sb = att_pool.tile([128, BATCH // N_CORES * (SRC_LEN // 128)], f32)

            # consts on the ACT hwdge queue, staged in first-consumption
            # order (hot chunks first) so the first group gates minimally
            for c in range(N_EC):
                nc.scalar.dma_start(w16_sb[:, c], w16[:, c])
            for j in range(N_DR):
                jsl = slice(2 * j, 2 * j + 2)
                nc.scalar.dma_start(w8_sb[:, jsl], w8[:, jsl])
            nc.scalar.dma_start(vb_sb[:], vb[:])

            cbb_sbs = [None] * B_LOC

            def load_cbb(b):
                t = cbb_pool.tile([128, HID], f32, tag="cbb", name=f"cbb_{b}")
                nc.scalar.dma_start(t[:], cbb[b])
                cbb_sbs[b] = t

            load_cbb(0)

            # warmup tanh for the ACT LUT-table dependency
            warm = const_pool.tile([128, 1], f32)
            nc.scalar.activation(
                warm[:], vb_sb[:, 0:1], mybir.ActivationFunctionType.Tanh
            )

            halves = [(b, h) for b in range(B_LOC) for h in range(2)]
            e16_sbs = {}
            e8_sbs = {}

            def alloc_half(i):
                e16_sbs[i] = e16_pool.tile(
                    [128, N_EC, SH], fp16, tag="e16", name=f"e16_{i}"
                )
                e8_sbs[i] = e8_pool.tile(
                    [128, N_EC, SH], fp8, tag="e8", name=f"e8_{i}"
                )

            def load_half(i):
                b, h = halves[i]
                nc.sync.dma_start(e16_sbs[i][:], enc16[b, h])
                nc.scalar.dma_start(e8_sbs[i][:], enc8[b, h])

            # first half in graduated s-slabs so the PE starts early
            alloc_half(0)
            s0 = 0
            for sw in (256, 256, 512):
                nc.sync.dma_start(
                    e16_sbs[0][:, :, s0:s0 + sw], enc16[0, 0, :, :, s0:s0 + sw]
                )
                nc.sync.dma_start(
                    e8_sbs[0][:, :, s0:s0 + sw], enc8[0, 0, :, :, s0:s0 + sw]
                )
                s0 += sw

            for i, (b, h) in enumerate(halves):
                for st in range(N_STH):
                    if i + 1 < len(halves) and st == 1:
                        alloc_half(i + 1)
                        load_half(i + 1)
                        if h == 1:
                            load_cbb(b + 1)
                    sl = slice(st * 128, (st + 1) * 128)
                    ps_h = psum_pool.tile(
                        [128, HOT], f32, tag="psh", name=f"psh_{i}_{st}"
                    )
                    ps_c0 = psum_pool.tile(
                        [128, C0], f32, tag="psc0", name=f"psc0_{i}_{st}"
                    )
                    ps_c1 = psum_pool.tile(
                        [128, C1], f32, tag="psc1", name=f"psc1_{i}_{st}"
                    )

                    # split LdWeights/Matmult: the next stationary (enc
                    # chunk) loads while the current moving phase streams
                    def mm(psum, rhs, start, stop, perf_mode=None):
                        inst = nc.tensor.matmul(
                            psum, lhsT=lhs, rhs=rhs,
                            start=start, stop=stop, perf_mode=perf_mode,
                        )
                        inst.ins.ldweights = False

                    for c in range(N_EC):
                        lhs = e16_sbs[i][:, c, sl]
                        nc.tensor.ldweights(lhs)
                        mm(ps_h[:], w16_sb[:, c], start=(c == 0),
                           stop=(c == N_EC - 1))
                    for j in range(N_DR):
                        lhs = e8_sbs[i][:, 2 * j:2 * j + 2, sl]
                        nc.tensor.ldweights(
                            lhs, perf_mode=mybir.MatmulPerfMode.DoubleRow
                        )
                        mm(ps_c0[:], w8_sb[:, 2 * j:2 * j + 2, 0:C0],
                           start=(j == 0), stop=(j == N_DR - 1),
                           perf_mode=mybir.MatmulPerfMode.DoubleRow)
                        mm(ps_c1[:], w8_sb[:, 2 * j:2 * j + 2, C0:COLD],
                           start=(j == 0), stop=(j == N_DR - 1),
                           perf_mode=mybir.MatmulPerfMode.DoubleRow)

                    tanhE = te_pool.tile(
                        [128, HID], fp16, tag="te", name=f"te_{i}_{st}"
                    )
                    for ps, lo, hi in (
                        (ps_h, 0, HOT),
                        (ps_c0, HOT, HOT + C0),
                        (ps_c1, HOT + C0, HID),
                    ):
                        pre = pre_pool.tile(
                            [128, hi - lo], f32, tag="pre",
                            name=f"pre_{i}_{st}_{lo}",
                        )
                        nc.vector.tensor_add(
                            out=pre[:], in0=ps[:], in1=cbb_sbs[b][:, lo:hi]
                        )
                        nc.scalar.activation(
                            tanhE[:, lo:hi], pre[:],
                            mybir.ActivationFunctionType.Tanh,
                            scale=1.0 / WS,
                        )
                    tt = tt_pool.tile(
                        [128, HID], fp16, tag="tt", name=f"tt_{i}_{st}"
                    )
                    nc.gpsimd.tensor_mul(out=tt[:], in0=tanhE[:], in1=vb_sb[:])
                    k = b * (SRC_LEN // 128) + h * N_STH + st
                    nc.vector.tensor_reduce(
                        att_sb[:, k:k + 1],
                        tt[:],
                        mybir.AxisListType.X,
                        mybir.AluOpType.add,
                    )
                if h == 1:
                    nst = SRC_LEN // 128
                    nc.scalar.dma_start(
                        att[b], att_sb[:, b * nst:(b + 1) * nst]
                    )
    nc.compile()
    return nc


def _get_nc():
    if "nc" not in _NC_CACHE:
        _NC_CACHE["nc"] = _build()
    return _NC_CACHE["nc"]


def kernel(hidden, encoder_outputs, attn_w, attn_b, v_w, _trace=False):
    hidden = np.asarray(hidden, dtype=np.float32)
    encoder_outputs = np.asarray(encoder_outputs, dtype=np.float32)
    attn_w = np.asarray(attn_w, dtype=np.float32)
    attn_b = np.asarray(attn_b, dtype=np.float32)
    v_w = np.asarray(v_w, dtype=np.float32)

    perm = np.argsort(-np.abs(v_w))                    # hot |v| first
    c_b = ((hidden @ attn_w[:, :HID].T + attn_b)[:, perm]) * WS
    w_e = attn_w[:, HID:][perm]                        # [H, E] permuted rows
    # [E, H] -> [chunk, e, h] -> partition-major [e, chunk, h]
    w_t = np.ascontiguousarray(
        (w_e.T * WS).reshape(N_EC, 128, HID).transpose(1, 0, 2)
    )
    w16_dev = w_t[:, :, :HOT].astype(np.float16)
    w8_dev = np.ascontiguousarray(w_t[:, :, HOT:]).astype(ml_dtypes.float8_e4m3)
    vb_dev = np.ascontiguousarray(
        np.broadcast_to(v_w[perm][None, :], (128, HID))
    ).astype(np.float16)

    nc = _get_nc()
    in_maps = []
    for core in range(N_CORES):
        b0 = core * B_LOC
        e16_rows = np.empty((B_LOC, 2, 128, N_EC, SH), dtype=np.float16)
        e8_rows = np.empty((B_LOC, 2, 128, N_EC, SH), dtype=ml_dtypes.float8_e4m3)
        for b in range(B_LOC):
            # enc[:, b, :] is [S, E]; make [half, e, chunk, s] lines
            ect = encoder_outputs[:, b0 + b, :].T.reshape(N_EC, 128, 2, SH)
            ect = ect.transpose(2, 1, 0, 3)
            e16_rows[b] = ect
            e8_rows[b] = ect
        cbb_dev = np.ascontiguousarray(
            np.broadcast_to(c_b[b0:b0 + B_LOC, None, :], (B_LOC, 128, HID))
        ).astype(np.float32)
        in_maps.append(
            {
                "enc16": e16_rows,
                "enc8": e8_rows,
                "w16": w16_dev,
                "w8": w8_dev,
                "cbb": cbb_dev,
                "vb": vb_dev,
            }
        )

    res = run_bass_kernel_spmd(
        nc, in_maps, core_ids=list(range(N_CORES)), trace=_trace
    )
    if _trace:
        _NC_CACHE["last_result"] = res

    att = np.concatenate(
        [
            res.results[c]["att"].transpose(0, 2, 1).reshape(B_LOC, SRC_LEN)
            for c in range(N_CORES)
        ],
        axis=0,
    )  # [B, S] logits

    m = att.max(axis=1, keepdims=True)
    e = np.exp(att - m)
    out = e / e.sum(axis=1, keepdims=True)
    return out.astype(np.float32)


# revision 24
# speedup vs baseline: 1.0683x; 1.0683x over previous
"""Trainium2 Bass kernel for nn_Attention (Bahdanau-style additive attention).

Reference computation:
    enc = encoder_outputs.transpose(1, 0, 2)            # [B, S, 2H]
    e_proj = enc @ w_e.T                                # [B, S, H]
    energy = tanh(h_proj[:, None, :] + e_proj + b)      # [B, S, H]
    att = energy @ v_w                                  # [B, S]
    out = softmax(att, axis=1)

Sharding: data-parallel over batch, 4 batch rows per core on 8 cores.

Per-core pipeline, |v|-stratified mixed precision: the logit error from
quantizing the e_proj GEMM is sum_h v_h * tanh' * dx_h, so the h
columns are permuted by descending |v_h| (host side) and the HOT
highest-|v| columns are computed in fp16 while the remaining COLD
columns run entirely in fp8 (e4m3) DoubleRow matmuls at 2x PE
throughput.  This buys the same accuracy as a chunk-wise fp16/fp8
split at ~12% less PE time.
  - enc is pre-transposed and quantized on the host into partition-major
    [e, chunk, s] lines (both dtypes cover all 16 contraction chunks),
    streamed per half batch row; all DMA is plain loads, no transpose
  - w_e and c_b are pre-scaled by WS=64 so the fp8 weights stay in the
    e4m3 normal range; the tanh activation applies scale=1/WS to undo it
  - LdWeights is split from Matmult so the next stationary (an enc
    chunk) loads while the current moving phase streams
  - epilogue per 128-position s-tile: DVE adds the broadcast bias
    c_b*WS per psum region, ACT applies tanh(x/WS), GPSIMD (otherwise
    idle) multiplies by v and reduces over h into the logit column
h_proj ([32,1024] @ [1024,1024]) and the final softmax over [32, 2048]
are tiny and run on the host in fp32.
"""

import sys

try:
    import concourse.bass as bass  # noqa: F401
except ImportError:
    sys.path.insert(0, "/opt/trn_rl_repo")

import numpy as np
import ml_dtypes

import concourse.bacc as bacc
import concourse.mybir as mybir
import concourse.tile as tile
from concourse.bass_utils import run_bass_kernel_spmd

HID = 1024
BATCH = 32
SRC_LEN = 2048

N_CORES = 8
B_LOC = BATCH // N_CORES      # 4
E = 2 * HID                   # 2048
N_EC = E // 128               # 16 e-chunks of 128
N_DR = N_EC // 2              # 8 fp8 DoubleRow chunk-pairs
HOT = 160                     # fp16 h-columns (highest |v|), permuted first
COLD = HID - HOT              # 832 fp8 h-columns
C0 = 512                      # cold psum region split: 512 + 320
C1 = COLD - C0                # 320
SH = SRC_LEN // 2             # 1024 s per half-row pipeline stage
N_STH = SH // 128             # 8 s-tiles per half
WS = 64.0                     # weight/bias pre-scale (fp8 range)

f32 = mybir.dt.float32
fp16 = mybir.dt.float16
fp8 = mybir.dt.float8e4

_NC_CACHE = {}


def _build():
    nc = bacc.Bacc(
        "TRN2", target_bir_lowering=False, debug=False, num_devices=N_CORES
    )
    enc16 = nc.declare_dram_parameter(
        "enc16", [B_LOC, 128, N_EC, SRC_LEN], fp16, isOutput=False
    )
    enc8 = nc.declare_dram_parameter(
        "enc8", [B_LOC, 128, N_EC, SRC_LEN], fp8, isOutput=False
    )
    w16 = nc.declare_dram_parameter("w16", [128, N_EC, HOT], fp16, isOutput=False)
    w8 = nc.declare_dram_parameter("w8", [128, N_EC, COLD], fp8, isOutput=False)
    cbb = nc.declare_dram_parameter("cbb", [B_LOC, 128, HID], f32, isOutput=False)
    vb = nc.declare_dram_parameter("vb", [128, HID], fp16, isOutput=False)
    # [b, p, st]: logit(b, st*128 + p) in permuted-h space (h only summed)
    att = nc.declare_dram_parameter(
        "att", [B_LOC, 128, SRC_LEN // 128], f32, isOutput=True
    )

    with tile.TileContext(nc) as tc:
        with (
            tc.tile_pool(name="const", bufs=1) as const_pool,
            tc.tile_pool(name="e16p", bufs=3) as e16_pool,
            tc.tile_pool(name="e8p", bufs=3) as e8_pool,
            tc.tile_pool(name="cbbp", bufs=2) as cbb_pool,
            tc.tile_pool(name="prep", bufs=4) as pre_pool,
            tc.tile_pool(name="tep", bufs=3) as te_pool,
            tc.tile_pool(name="ttp", bufs=2) as tt_pool,
            tc.tile_pool(name="attsb", bufs=1) as att_pool,
            tc.tile_pool(name="psum", bufs=2, space="PSUM") as psum_pool,
        ):
            w16_sb = const_pool.tile([128, N_EC, HOT], fp16)
            w8_sb = const_pool.tile([128, N_EC, COLD], fp8)
            vb_sb = const_pool.tile([128, HID], fp16)
            att_sb = att_pool.tile([128, BATCH // N_CORES * (SRC_LEN // 128)], f32)

            # consts on the ACT hwdge queue, staged in first-consumption
            # order (hot chunks first) so the first group gates minimally
            for c in range(N_EC):
                nc.scalar.dma_start(w16_sb[:, c], w16[:, c])
            for j in range(N_DR):
                jsl = slice(2 * j, 2 * j + 2)
                nc.scalar.dma_start(w8_sb[:, jsl], w8[:, jsl])
            nc.scalar.dma_start(vb_sb[:], vb[:])

            cbb_sbs = [None] * B_LOC

            def load_cbb(b):
                t = cbb_pool.tile([128, HID], f32, tag="cbb", name=f"cbb_{b}")
                nc.scalar.dma_start(t[:], cbb[b])
                cbb_sbs[b] = t

            load_cbb(0)

            # warmup tanh for the ACT LUT-table dependency
            warm = const_pool.tile([128, 1], f32)
            nc.scalar.activation(
                warm[:], vb_sb[:, 0:1], mybir.ActivationFunctionType.Tanh
            )

            halves = [(b, h) for b in range(B_LOC) for h in range(2)]
            e16_sbs = {}
            e8_sbs = {}

            def alloc_half(i):
                e16_sbs[i] = e16_pool.tile(
                    [128, N_EC, SH], fp16, tag="e16", name=f"e16_{i}"
                )
                e8_sbs[i] = e8_pool.tile(
                    [128, N_EC, SH], fp8, tag="e8", name=f"e8_{i}"
                )

            def load_half(i):
                b, h = halves[i]
                ssl = slice(h * SH, (h + 1) * SH)
                nc.sync.dma_start(e16_sbs[i][:], enc16[b, :, :, ssl])
                nc.sync.dma_start(e8_sbs[i][:], enc8[b, :, :, ssl])

            # first half in graduated s-slabs so the PE starts early
            alloc_half(0)
            s0 = 0
            for sw in (256, 256, 512):
                nc.sync.dma_start(
                    e16_sbs[0][:, :, s0:s0 + sw], enc16[0, :, :, s0:s0 + sw]
                )
                nc.sync.dma_start(
                    e8_sbs[0][:, :, s0:s0 + sw], enc8[0, :, :, s0:s0 + sw]
                )
                s0 += sw

            for i, (b, h) in enumerate(halves):
                for st in range(N_STH):
                    if i + 1 < len(halves) and st == 1:
                        alloc_half(i + 1)
                        load_half(i + 1)
                        if h == 1:
                            load_cbb(b + 1)
                    sl = slice(st * 128, (st + 1) * 128)
                    ps_h = psum_pool.tile(
                        [128, HOT], f32, tag="psh", name=f"psh_{i}_{st}"
                    )
                    ps_c0 = psum_pool.tile(
                        [128, C0], f32, tag="psc0", name=f"psc0_{i}_{st}"
                    )
                    ps_c1 = psum_pool.tile(
                        [128, C1], f32, tag="psc1", name=f"psc1_{i}_{st}"
                    )

                    # split LdWeights/Matmult: the next stationary (enc
                    # chunk) loads while the current moving phase streams
                    def mm(psum, rhs, start, stop, perf_mode=None):
                        inst = nc.tensor.matmul(
                            psum, lhsT=lhs, rhs=rhs,
                            start=start, stop=stop, perf_mode=perf_mode,
                        )
                        inst.ins.ldweights = False

                    for c in range(N_EC):
                        lhs = e16_sbs[i][:, c, sl]
                        nc.tensor.ldweights(lhs)
                        mm(ps_h[:], w16_sb[:, c], start=(c == 0),
                           stop=(c == N_EC - 1))
                    for j in range(N_DR):
                        lhs = e8_sbs[i][:, 2 * j:2 * j + 2, sl]
                        nc.tensor.ldweights(
                            lhs, perf_mode=mybir.MatmulPerfMode.DoubleRow
                        )
                        mm(ps_c0[:], w8_sb[:, 2 * j:2 * j + 2, 0:C0],
                           start=(j == 0), stop=(j == N_DR - 1),
                           perf_mode=mybir.MatmulPerfMode.DoubleRow)
                        mm(ps_c1[:], w8_sb[:, 2 * j:2 * j + 2, C0:COLD],
                           start=(j == 0), stop=(j == N_DR - 1),
                           perf_mode=mybir.MatmulPerfMode.DoubleRow)

                    tanhE = te_pool.tile(
                        [128, HID], fp16, tag="te", name=f"te_{i}_{st}"
                    )
                    for ps, lo, hi in (
                        (ps_h, 0, HOT),
                        (ps_c0, HOT, HOT + C0),
                        (ps_c1, HOT + C0, HID),
                    ):
                        pre = pre_pool.tile(
                            [128, hi - lo], f32, tag="pre",
                            name=f"pre_{i}_{st}_{lo}",
                        )
                        nc.vector.tensor_add(
                            out=pre[:], in0=ps[:], in1=cbb_sbs[b][:, lo:hi]
                        )
                        nc.scalar.activation(
                            tanhE[:, lo:hi], pre[:],
                            mybir.ActivationFunctionType.Tanh,
                            scale=1.0 / WS,
                        )
                    tt = tt_pool.tile(
                        [128, HID], fp16, tag="tt", name=f"tt_{i}_{st}"
                    )
                    nc.gpsimd.tensor_mul(out=tt[:], in0=tanhE[:], in1=vb_sb[:])
                    k = b * (SRC_LEN // 128) + h * N_STH + st
                    nc.vector.tensor_reduce(
                        att_sb[:, k:k + 1],
                        tt[:],
                        mybir.AxisListType.X,
                        mybir.AluOpType.add,
                    )
                if h == 1:
                    nst = SRC_LEN // 128
                    nc.scalar.dma_start(
                        att[b], att_sb[:, b * nst:(b + 1) * nst]
                    )
    nc.compile()
    return nc


def _get_nc():
    if "nc" not in _NC_CACHE:
        _NC_CACHE["nc"] = _build()
    return _NC_CACHE["nc"]


def kernel(hidden, encoder_outputs, attn_w, attn_b, v_w, _trace=False):
    hidden = np.asarray(hidden, dtype=np.float32)
    encoder_outputs = np.asarray(encoder_outputs, dtype=np.float32)
    attn_w = np.asarray(attn_w, dtype=np.float32)
    attn_b = np.asarray(attn_b, dtype=np.float32)
    v_w = np.asarray(v_w, dtype=np.float32)

    perm = np.argsort(-np.abs(v_w))                    # hot |v| first
    c_b = ((hidden @ attn_w[:, :HID].T + attn_b)[:, perm]) * WS
    w_e = attn_w[:, HID:][perm]                        # [H, E] permuted rows
    # [E, H] -> [chunk, e, h] -> partition-major [e, chunk, h]
    w_t = np.ascontiguousarray(
        (w_e.T * WS).reshape(N_EC, 128, HID).transpose(1, 0, 2)
    )
    w16_dev = w_t[:, :, :HOT].astype(np.float16)
    w8_dev = np.ascontiguousarray(w_t[:, :, HOT:]).astype(ml_dtypes.float8_e4m3)
    vb_dev = np.ascontiguousarray(
        np.broadcast_to(v_w[perm][None, :], (128, HID))
    ).astype(np.float16)

    nc = _get_nc()
    in_maps = []
    for core in range(N_CORES):
        b0 = core * B_LOC
        e16_rows = np.empty((B_LOC, 128, N_EC, SRC_LEN), dtype=np.float16)
        e8_rows = np.empty((B_LOC, 128, N_EC, SRC_LEN), dtype=ml_dtypes.float8_e4m3)
        for b in range(B_LOC):
            # enc[:, b, :] is [S, E]; make [e, chunk, s] lines
            ect = encoder_outputs[:, b0 + b, :].T.reshape(N_EC, 128, SRC_LEN)
            ect = ect.transpose(1, 0, 2)
            e16_rows[b] = ect
            e8_rows[b] = ect
        cbb_dev = np.ascontiguousarray(
            np.broadcast_to(c_b[b0:b0 + B_LOC, None, :], (B_LOC, 128, HID))
        ).astype(np.float32)
        in_maps.append(
            {
                "enc16": e16_rows,
                "enc8": e8_rows,
                "w16": w16_dev,
                "w8": w8_dev,
                "cbb": cbb_dev,
                "vb": vb_dev,
            }
        )

    res = run_bass_kernel_spmd(
        nc, in_maps, core_ids=list(range(N_CORES)), trace=_trace
    )
    if _trace:
        _NC_CACHE["last_result"] = res

    att = np.concatenate(
        [
            res.results[c]["att"].transpose(0, 2, 1).reshape(B_LOC, SRC_LEN)
            for c in range(N_CORES)
        ],
        axis=0,
    )  # [B, S] logits

    m = att.max(axis=1, keepdims=True)
    e = np.exp(att - m)
    out = e / e.sum(axis=1, keepdims=True)
    return out.astype(np.float32)


# revision 26
# speedup vs baseline: 1.4326x; 1.3410x over previous
"""Trainium2 Bass kernel for nn_Attention (Bahdanau-style additive attention).

Reference computation:
    enc = encoder_outputs.transpose(1, 0, 2)            # [B, S, 2H]
    e_proj = enc @ w_e.T                                # [B, S, H]
    energy = tanh(h_proj[:, None, :] + e_proj + b)      # [B, S, H]
    att = energy @ v_w                                  # [B, S]
    out = softmax(att, axis=1)

Sharding: data-parallel over batch, 4 batch rows per core on 8 cores.

Per-core pipeline, |v|-stratified mixed precision: the logit error from
quantizing the e_proj GEMM is sum_h v_h * tanh' * dx_h, so the h
columns are permuted by descending |v_h| (host side) and the HOT
highest-|v| columns are computed in fp16 while the remaining COLD
columns run entirely in fp8 (e4m3) DoubleRow matmuls at 2x PE
throughput.  This buys the same accuracy as a chunk-wise fp16/fp8
split at ~12% less PE time.
  - enc is pre-transposed and quantized on the host into partition-major
    [e, chunk, s] lines (both dtypes cover all 16 contraction chunks),
    streamed per half batch row; all DMA is plain loads, no transpose
  - w_e and c_b are pre-scaled by WS=64 so the fp8 weights stay in the
    e4m3 normal range; the tanh activation applies scale=1/WS to undo it
  - LdWeights is split from Matmult so the next stationary (an enc
    chunk) loads while the current moving phase streams
  - epilogue per 128-position s-tile: DVE adds the broadcast bias
    c_b*WS per psum region, ACT applies tanh(x/WS), GPSIMD (otherwise
    idle) multiplies by v and reduces over h into the logit column
h_proj ([32,1024] @ [1024,1024]) and the final softmax over [32, 2048]
are tiny and run on the host in fp32.
"""

import sys

try:
    import concourse.bass as bass  # noqa: F401
except ImportError:
    sys.path.insert(0, "/opt/trn_rl_repo")

import numpy as np
import ml_dtypes

import concourse.bacc as bacc
import concourse.mybir as mybir
import concourse.tile as tile
from concourse.bass_utils import run_bass_kernel_spmd

HID = 1024
BATCH = 32
SRC_LEN = 2048

N_CORES = 8
B_LOC = BATCH // N_CORES      # 4
E = 2 * HID                   # 2048
N_EC = E // 128               # 16 e-chunks of 128
N_DR = N_EC // 2              # 8 fp8 DoubleRow chunk-pairs
HOT = 192                     # fp16 h-columns (highest |v|), permuted first
COLD = HID - HOT              # 832 fp8 h-columns
C0 = 512                      # cold psum region split: 512 + 320
C1 = COLD - C0                # 320
SH = SRC_LEN // 2             # 1024 s per half-row pipeline stage
N_STH = SH // 128             # 8 s-tiles per half
WS = 64.0                     # weight/bias pre-scale (fp8 range)

f32 = mybir.dt.float32
fp16 = mybir.dt.float16
fp8 = mybir.dt.float8e4

_NC_CACHE = {}


def _build():
    nc = bacc.Bacc(
        "TRN2", target_bir_lowering=False, debug=False, num_devices=N_CORES
    )
    enc16 = nc.declare_dram_parameter(
        "enc16", [B_LOC, 128, N_EC, SRC_LEN], fp16, isOutput=False
    )
    enc8 = nc.declare_dram_parameter(
        "enc8", [B_LOC, 128, N_EC, SRC_LEN], fp8, isOutput=False
    )
    w16 = nc.declare_dram_parameter("w16", [128, N_EC, HOT], fp16, isOutput=False)
    w8 = nc.declare_dram_parameter("w8", [128, N_EC, COLD], fp8, isOutput=False)
    cbb = nc.declare_dram_parameter("cbb", [B_LOC, 128, HID], f32, isOutput=False)
    vb = nc.declare_dram_parameter("vb", [128, HID], fp16, isOutput=False)
    # [b, p, st]: logit(b, st*128 + p) in permuted-h space (h only summed)
    att = nc.declare_dram_parameter(
        "att", [B_LOC, 128, SRC_LEN // 128], f32, isOutput=True
    )

    with tile.TileContext(nc) as tc:
        with (
            tc.tile_pool(name="const", bufs=1) as const_pool,
            tc.tile_pool(name="e16p", bufs=2) as e16_pool,
            tc.tile_pool(name="e8p", bufs=2) as e8_pool,
            tc.tile_pool(name="cbbp", bufs=2) as cbb_pool,
            tc.tile_pool(name="prep", bufs=4) as pre_pool,
            tc.tile_pool(name="tep", bufs=3) as te_pool,
            tc.tile_pool(name="ttp", bufs=2) as tt_pool,
            tc.tile_pool(name="attsb", bufs=1) as att_pool,
            tc.tile_pool(name="psum", bufs=2, space="PSUM") as psum_pool,
        ):
            w16_sb = const_pool.tile([128, N_EC, HOT], fp16)
            w8_sb = const_pool.tile([128, N_EC, COLD], fp8)
            vb_sb = const_pool.tile([128, HID], fp16)
            att_sb = att_pool.tile([128, BATCH // N_CORES * (SRC_LEN // 128)], f32)

            # consts on the ACT hwdge queue, staged in first-consumption
            # order (hot chunks first) so the first group gates minimally
            for c in range(N_EC):
                nc.scalar.dma_start(w16_sb[:, c], w16[:, c])
            for j in range(N_DR):
                jsl = slice(2 * j, 2 * j + 2)
                nc.scalar.dma_start(w8_sb[:, jsl], w8[:, jsl])
            nc.scalar.dma_start(vb_sb[:], vb[:])

            cbb_sbs = [None] * B_LOC

            def load_cbb(b):
                t = cbb_pool.tile([128, HID], f32, tag="cbb", name=f"cbb_{b}")
                nc.scalar.dma_start(t[:], cbb[b])
                cbb_sbs[b] = t

            load_cbb(0)

            # warmup tanh for the ACT LUT-table dependency
            warm = const_pool.tile([128, 1], f32)
            nc.scalar.activation(
                warm[:], vb_sb[:, 0:1], mybir.ActivationFunctionType.Tanh
            )

            halves = [(b, h) for b in range(B_LOC) for h in range(2)]
            e16_sbs = {}
            e8_sbs = {}

            def alloc_half(i):
                e16_sbs[i] = e16_pool.tile(
                    [128, N_EC, SH], fp16, tag="e16", name=f"e16_{i}"
                )
                e8_sbs[i] = e8_pool.tile(
                    [128, N_EC, SH], fp8, tag="e8", name=f"e8_{i}"
                )

            def load_half(i):
                b, h = halves[i]
                ssl = slice(h * SH, (h + 1) * SH)
                nc.sync.dma_start(e16_sbs[i][:], enc16[b, :, :, ssl])
                nc.sync.dma_start(e8_sbs[i][:], enc8[b, :, :, ssl])

            # first half in graduated s-slabs so the PE starts early
            alloc_half(0)
            s0 = 0
            for sw in (256, 256, 512):
                nc.sync.dma_start(
                    e16_sbs[0][:, :, s0:s0 + sw], enc16[0, :, :, s0:s0 + sw]
                )
                nc.sync.dma_start(
                    e8_sbs[0][:, :, s0:s0 + sw], enc8[0, :, :, s0:s0 + sw]
                )
                s0 += sw

            for i, (b, h) in enumerate(halves):
                for st in range(N_STH):
                    if i + 1 < len(halves) and st == 1:
                        alloc_half(i + 1)
                        load_half(i + 1)
                        if h == 1:
                            load_cbb(b + 1)
                    sl = slice(st * 128, (st + 1) * 128)
                    ps_h = psum_pool.tile(
                        [128, HOT], f32, tag="psh", name=f"psh_{i}_{st}"
                    )
                    ps_c0 = psum_pool.tile(
                        [128, C0], f32, tag="psc0", name=f"psc0_{i}_{st}"
                    )
                    ps_c1 = psum_pool.tile(
                        [128, C1], f32, tag="psc1", name=f"psc1_{i}_{st}"
                    )

                    # split LdWeights/Matmult: the next stationary (enc
                    # chunk) loads while the current moving phase streams
                    def mm(psum, rhs, start, stop, perf_mode=None):
                        inst = nc.tensor.matmul(
                            psum, lhsT=lhs, rhs=rhs,
                            start=start, stop=stop, perf_mode=perf_mode,
                        )
                        inst.ins.ldweights = False

                    for c in range(N_EC):
                        lhs = e16_sbs[i][:, c, sl]
                        nc.tensor.ldweights(lhs)
                        mm(ps_h[:], w16_sb[:, c], start=(c == 0),
                           stop=(c == N_EC - 1))
                    for j in range(N_DR):
                        lhs = e8_sbs[i][:, 2 * j:2 * j + 2, sl]
                        nc.tensor.ldweights(
                            lhs, perf_mode=mybir.MatmulPerfMode.DoubleRow
                        )
                        mm(ps_c0[:], w8_sb[:, 2 * j:2 * j + 2, 0:C0],
                           start=(j == 0), stop=(j == N_DR - 1),
                           perf_mode=mybir.MatmulPerfMode.DoubleRow)
                        mm(ps_c1[:], w8_sb[:, 2 * j:2 * j + 2, C0:COLD],
                           start=(j == 0), stop=(j == N_DR - 1),
                           perf_mode=mybir.MatmulPerfMode.DoubleRow)

                    tanhE = te_pool.tile(
                        [128, HID], fp16, tag="te", name=f"te_{i}_{st}"
                    )
                    for ps, lo, hi in (
                        (ps_h, 0, HOT),
                        (ps_c0, HOT, HOT + C0),
                        (ps_c1, HOT + C0, HID),
                    ):
                        pre = pre_pool.tile(
                            [128, hi - lo], f32, tag="pre",
                            name=f"pre_{i}_{st}_{lo}",
                        )
                        nc.vector.tensor_add(
                            out=pre[:], in0=ps[:], in1=cbb_sbs[b][:, lo:hi]
                        )
                        nc.scalar.activation(
                            tanhE[:, lo:hi], pre[:],
                            mybir.ActivationFunctionType.Tanh,
                            scale=1.0 / WS,
                        )
                    tt = tt_pool.tile(
                        [128, HID], fp16, tag="tt", name=f"tt_{i}_{st}"
                    )
                    nc.gpsimd.tensor_mul(out=tt[:], in0=tanhE[:], in1=vb_sb[:])
                    k = b * (SRC_LEN // 128) + h * N_STH + st
                    nc.vector.tensor_reduce(
                        att_sb[:, k:k + 1],
                        tt[:],
                        mybir.AxisListType.X,
                        mybir.AluOpType.add,
                    )
                if h == 1:
                    nst = SRC_LEN // 128
                    nc.scalar.dma_start(
                        att[b], att_sb[:, b * nst:(b + 1) * nst]
                    )
    nc.compile()
    return nc


def _get_nc():
    if "nc" not in _NC_CACHE:
        _NC_CACHE["nc"] = _build()
    return _NC_CACHE["nc"]


def kernel(hidden, encoder_outputs, attn_w, attn_b, v_w, _trace=False):
    hidden = np.asarray(hidden, dtype=np.float32)
    encoder_outputs = np.asarray(encoder_outputs, dtype=np.float32)
    attn_w = np.asarray(attn_w, dtype=np.float32)
    attn_b = np.asarray(attn_b, dtype=np.float32)
    v_w = np.asarray(v_w, dtype=np.float32)

    perm = np.argsort(-np.abs(v_w))                    # hot |v| first
    c_b = ((hidden @ attn_w[:, :HID].T + attn_b)[:, perm]) * WS
    w_e = attn_w[:, HID:][perm]                        # [H, E] permuted rows
    # [E, H] -> [chunk, e, h] -> partition-major [e, chunk, h]
    w_t = np.ascontiguousarray(
        (w_e.T * WS).reshape(N_EC, 128, HID).transpose(1, 0, 2)
    )
    w16_dev = w_t[:, :, :HOT].astype(np.float16)
    w8_dev = np.ascontiguousarray(w_t[:, :, HOT:]).astype(ml_dtypes.float8_e4m3)
    vb_dev = np.ascontiguousarray(
        np.broadcast_to(v_w[perm][None, :], (128, HID))
    ).astype(np.float16)

    nc = _get_nc()
    in_maps = []
    for core in range(N_CORES):
        b0 = core * B_LOC
        e16_rows = np.empty((B_LOC, 128, N_EC, SRC_LEN), dtype=np.float16)
        e8_rows = np.empty((B_LOC, 128, N_EC, SRC_LEN), dtype=ml_dtypes.float8_e4m3)
        for b in range(B_LOC):
            # enc[:, b, :] is [S, E]; make [e, chunk, s] lines
            ect = encoder_outputs[:, b0 + b, :].T.reshape(N_EC, 128, SRC_LEN)
            ect = ect.transpose(1, 0, 2)
            e16_rows[b] = ect
            e8_rows[b] = ect
        cbb_dev = np.ascontiguousarray(
            np.broadcast_to(c_b[b0:b0 + B_LOC, None, :], (B_LOC, 128, HID))
        ).astype(np.float32)
        in_maps.append(
            {
                "enc16": e16_rows,
                "enc8": e8_rows,
                "w16": w16_dev,
                "w8": w8_dev,
                "cbb": cbb_dev,
                "vb": vb_dev,
            }
        )

    res = run_bass_kernel_spmd(
        nc, in_maps, core_ids=list(range(N_CORES)), trace=_trace
    )
    if _trace:
        _NC_CACHE["last_result"] = res

    att = np.concatenate(
        [
            res.results[c]["att"].transpose(0, 2, 1).reshape(B_LOC, SRC_LEN)
            for c in range(N_CORES)
        ],
        axis=0,
    )  # [B, S] logits

    m = att.max(axis=1, keepdims=True)
    e = np.exp(att - m)
    out = e / e.sum(axis=1, keepdims=True)
    return out.astype(np.float32)
